# revision 13
# baseline (speedup 1.0000x reference)
"""LocationRelativeAttention Trainium2 kernel.

Full-input contract: kernel(**inputs) takes the unsharded numpy inputs and
returns (context[64,256], cum[64,4096], alignment_full[64,4096], new_ws[64]).

Sharding: pure data-parallel over batch (64 -> 8 cores x 8 batches).
Per-core device kernel (Bass/Tile):
  - builds the padded cumulative-alignment sequence `loc`, bounces it to DRAM
  - im2col sliding-window DMA + PE matmul for the K=31 conv over `loc`
  - q = query @ wq.T (+bq+conv_b fused as tanh bias), feat = tanh(conv + q)
  - score = ws . feat via PE reduction, batched softmax on [8,1024]
  - context = alignment-weighted sum of the token window (PE, window gathered
    by dynamic-offset DMAs from the [B,T,H]-layout tokens shard)
  - alignment scatter, cum = ca + alignment_full, argmax/new_ws on device
"""
import sys

import numpy as np

try:
    import concourse.bass as bass
except ImportError:  # fresh grading dir: make the concourse repo importable
    for p in ("/opt/trn_rl_repo", "/root/.axon_site/_ro/trn_rl_repo"):
        if p not in sys.path:
            sys.path.insert(0, p)
    import concourse.bass as bass

import concourse.mybir as mybir
import concourse.tile as tile
from concourse import bacc
from concourse.masks import make_identity

F32 = mybir.dt.float32
I32 = mybir.dt.int32
AF = mybir.ActivationFunctionType
OP = mybir.AluOpType

T = 4096
B = 8            # batches per core
NCORES = 8
H = 256
QD = 1024
KF = 31
PAD = (KF - 1) // 2
WL = 1024
LOCW = T + 2 * PAD   # 4126
NK = WL // 128       # 8 w-chunks of 128 for the context contraction
NWC = WL // 512      # 2 w-chunks of 512 for conv/score
NHC = H // 128       # 2 h-chunks


def build_program():
    nc = bacc.Bacc("TRN2", target_bir_lowering=False, debug=False,
                   enable_asserts=False)

    tokens = nc.dram_tensor("tokens", [B, T, H], F32, kind="ExternalInput")
    cum_in = nc.dram_tensor("cum_in", [B, T], F32, kind="ExternalInput")
    init_ca = nc.dram_tensor("init_ca", [B, 1], F32, kind="ExternalInput")
    wstart = nc.dram_tensor("wstart", [B], I32, kind="ExternalInput")
    ntok = nc.dram_tensor("ntok", [B], I32, kind="ExternalInput")
    queryT = nc.dram_tensor("queryT", [QD, B], F32, kind="ExternalInput")
    wqT = nc.dram_tensor("wqT", [QD, H], F32, kind="ExternalInput")
    convwT = nc.dram_tensor("convwT", [KF, H], F32, kind="ExternalInput")
    qbias = nc.dram_tensor("qbias", [H], F32, kind="ExternalInput")
    wsvec = nc.dram_tensor("wsvec", [H], F32, kind="ExternalInput")

    context = nc.dram_tensor("context", [B, H], F32, kind="ExternalOutput")
    cum_out = nc.dram_tensor("cum_out", [B, T], F32, kind="ExternalOutput")
    afull = nc.dram_tensor("align_full", [B, T], F32, kind="ExternalOutput")
    new_ws = nc.dram_tensor("new_ws", [B], I32, kind="ExternalOutput")

    locd = nc.dram_tensor("loc_scratch", [B, LOCW], F32)

    with tile.TileContext(nc) as tc:
        with tc.tile_pool(name="const", bufs=1) as cpool, \
             tc.tile_pool(name="work", bufs=1) as wpool, \
             tc.tile_pool(name="toks", bufs=B) as tokpool, \
             tc.tile_pool(name="xw", bufs=3) as xpool, \
             tc.tile_pool(name="feat", bufs=4) as featpool, \
             tc.tile_pool(name="stage", bufs=2) as stagepool, \
             tc.tile_pool(name="small", bufs=1) as spool, \
             tc.tile_pool(name="dscratch", bufs=1, space="DRAM") as dram_pool, \
             tc.tile_pool(name="qps", bufs=1, space="PSUM") as q_ps, \
             tc.tile_pool(name="convps", bufs=2, space="PSUM") as conv_ps, \
             tc.tile_pool(name="scoreps", bufs=2, space="PSUM") as score_ps, \
             tc.tile_pool(name="trps", bufs=1, space="PSUM") as tr_ps, \
             tc.tile_pool(name="ctxps", bufs=2, space="PSUM") as ctx_ps:

            # ---- constant / parameter loads ----
            convwT_sb = cpool.tile([KF, H], F32)
            nc.sync.dma_start(convwT_sb[:, :], convwT.ap())
            wqT_sb = cpool.tile([128, (QD // 128) * H], F32)
            for c in range(QD // 128):
                nc.sync.dma_start(wqT_sb[:, c * H:(c + 1) * H],
                                  wqT.ap()[c * 128:(c + 1) * 128, :])
            qbias_sb = cpool.tile([128, NHC], F32)
            ws_sb = cpool.tile([128, NHC], F32)
            for hc in range(NHC):
                nc.sync.dma_start(qbias_sb[:, hc:hc + 1],
                                  qbias.ap()[hc * 128:(hc + 1) * 128])
                nc.sync.dma_start(ws_sb[:, hc:hc + 1],
                                  wsvec.ap()[hc * 128:(hc + 1) * 128])
            qTin_sb = cpool.tile([128, (QD // 128) * B], F32)
            for c in range(QD // 128):
                nc.sync.dma_start(qTin_sb[:, c * B:(c + 1) * B],
                                  queryT.ap()[c * 128:(c + 1) * 128, :])
            wsrow = cpool.tile([1, B], I32)
            nc.sync.dma_start(wsrow[:, :], wstart.ap())
            wscol = cpool.tile([B, 1], I32)
            nc.sync.dma_start(wscol[:, :], wstart.ap())
            ntcol = cpool.tile([B, 1], I32)
            nc.sync.dma_start(ntcol[:, :], ntok.ap())
            init_sb = cpool.tile([B, 1], F32)
            nc.sync.dma_start(init_sb[:, :], init_ca.ap())
            ident = cpool.tile([128, 128], F32)
            make_identity(nc, ident[:, :])
            revi = cpool.tile([B, WL], I32)
            nc.gpsimd.iota(revi[:, :], pattern=[[-1, WL]], base=T,
                           channel_multiplier=0)
            revf = cpool.tile([B, WL], F32)
            nc.vector.tensor_copy(revf[:, :], revi[:, :])

            # ---- loc = [init*PAD, ca, 0*PAD] ----
            loc_sb = wpool.tile([B, LOCW], F32)
            nc.sync.dma_start(loc_sb[:, PAD:PAD + T], cum_in.ap())
            nc.gpsimd.memset(loc_sb[:, PAD + T:], 0.0)
            nc.vector.tensor_copy(loc_sb[:, 0:PAD],
                                  init_sb[:, 0:1].to_broadcast([B, PAD]))
            nc.sync.dma_start(locd.ap(), loc_sb[:, :])

            # ---- window-start registers ----
            wsv = [nc.values_load(wsrow[0:1, b:b + 1], min_val=0,
                                  max_val=T - WL,
                                  skip_runtime_bounds_check=True)
                   for b in range(B)]

            # ---- token-window gather DMAs (the dominant HBM traffic) ----
            # One [WL, H] window per batch, viewed as [128, NK, H] so chunk k
            # is rows k*128..(k+1)*128 of the window (matmul rhs layout).
            tok_tiles = {}
            for b in range(B):
                t = tokpool.tile([128, NK * H], F32)
                win = tokens.ap()[b][bass.ds(wsv[b], WL), :]
                win = win.rearrange("(k p) h -> p k h", p=128)
                nc.sync.dma_start(t[:, :].rearrange("p (k h) -> p k h", k=NK),
                                  win)
                tok_tiles[b] = t

            # ---- q projection: qT[h, b] = sum_q wq[h,q] query[b,q] ----
            pq = q_ps.tile([128, NHC * B], F32)
            for hc in range(NHC):
                for c in range(QD // 128):
                    nc.tensor.matmul(
                        pq[:, hc * B:(hc + 1) * B],
                        lhsT=wqT_sb[:, c * H + hc * 128: c * H + hc * 128 + 128],
                        rhs=qTin_sb[:, c * B:(c + 1) * B],
                        start=(c == 0), stop=(c == QD // 128 - 1))
            qb_sb = cpool.tile([128, NHC * B], F32)
            for hc in range(NHC):
                nc.vector.tensor_scalar(qb_sb[:, hc * B:(hc + 1) * B],
                                        pq[:, hc * B:(hc + 1) * B],
                                        qbias_sb[:, hc:hc + 1], None, OP.add)

            # ---- conv + tanh + score, per batch ----
            loc_base = locd.ap()
            score_sb = wpool.tile([B, WL], F32)
            for b in range(B):
                X = xpool.tile([KF, WL], F32)
                src = bass.AP(tensor=loc_base.tensor,
                              offset=wsv[b] + b * LOCW,
                              ap=[[1, KF], [1, WL]])
                nc.sync.dma_start(X[:, :], src)
                st = stagepool.tile([1, WL], F32)
                for wc in range(NWC):
                    ps_s = score_ps.tile([1, 512], F32)
                    for hc in range(NHC):
                        pc = conv_ps.tile([128, 512], F32)
                        nc.tensor.matmul(
                            pc[:, :],
                            lhsT=convwT_sb[:, hc * 128:(hc + 1) * 128],
                            rhs=X[:, wc * 512:(wc + 1) * 512],
                            start=True, stop=True)
                        feat = featpool.tile([128, 512], F32)
                        nc.scalar.activation(
                            feat[:, :], pc[:, :], AF.Tanh,
                            bias=qb_sb[:, hc * B + b: hc * B + b + 1],
                            scale=1.0)
                        nc.tensor.matmul(
                            ps_s[0:1, :],
                            lhsT=ws_sb[:, hc:hc + 1], rhs=feat[:, :],
                            start=(hc == 0), stop=(hc == NHC - 1))
                    nc.scalar.copy(st[0:1, wc * 512:(wc + 1) * 512],
                                   ps_s[0:1, :])
                nc.sync.dma_start(score_sb[b:b + 1, :], st[0:1, :])

            # ---- batched softmax over the window ----
            negmax = spool.tile([B, 1], F32)
            nc.vector.tensor_reduce(negmax[:, :], score_sb[:, :],
                                    axis=mybir.AxisListType.X, op=OP.max,
                                    negate=True)
            align_e = wpool.tile([B, WL], F32)
            sume = spool.tile([B, 1], F32)
            nc.scalar.activation(align_e[:, :], score_sb[:, :], AF.Exp,
                                 bias=negmax[:, 0:1], scale=1.0,
                                 accum_out=sume[:, 0:1])
            rsum = spool.tile([B, 1], F32)
            nc.vector.reciprocal(rsum[:, :], sume[:, :])
            align_n = wpool.tile([B, WL], F32)
            nc.vector.tensor_scalar(align_n[:, :], align_e[:, :],
                                    rsum[:, 0:1], None, OP.mult)

            # ---- transpose alignment to [w, (k b)] for context lhsT ----
            ptr = tr_ps.tile([128, NK * B], F32)
            for k in range(NK):
                nc.tensor.transpose(ptr[:, k * B:(k + 1) * B],
                                    align_n[:, k * 128:(k + 1) * 128],
                                    ident[0:B, 0:B])
            al_T = wpool.tile([128, NK * B], F32)
            nc.vector.tensor_copy(al_T[:, :], ptr[:, :])

            # ---- context = sum_w align * tokens_window ----
            for b in range(B):
                pctx = ctx_ps.tile([1, H], F32)
                for k in range(NK):
                    nc.tensor.matmul(pctx[0:1, :],
                                     lhsT=al_T[:, k * B + b: k * B + b + 1],
                                     rhs=tok_tiles[b][:, k * H:(k + 1) * H],
                                     start=(k == 0), stop=(k == NK - 1))
                cst = stagepool.tile([1, H], F32, tag="cstage")
                nc.scalar.copy(cst[0:1, :], pctx[0:1, :])
                nc.sync.dma_start(context.ap()[b:b + 1, :], cst[0:1, :])

            # ---- alignment_full scatter (via DRAM, SBUF cannot take
            # partition+register dest offsets) + cum ----
            zt = wpool.tile([B, T], F32)
            nc.gpsimd.memset(zt[:, :], 0.0)
            af_dram = dram_pool.tile([B, T], F32)
            nc.sync.dma_start(af_dram[:, :], zt[:, :])
            for b in range(B):
                nc.sync.dma_start(af_dram[b:b + 1, bass.ds(wsv[b], WL)],
                                  align_n[b:b + 1, :])
            nc.sync.dma_start(afull.ap(), af_dram[:, :])
            af2 = wpool.tile([B, T], F32)
            nc.sync.dma_start(af2[:, :], af_dram[:, :])
            cumt = wpool.tile([B, T], F32)
            nc.vector.tensor_tensor(cumt[:, :], loc_sb[:, PAD:PAD + T],
                                    af2[:, :], op=OP.add)
            nc.sync.dma_start(cum_out.ap(), cumt[:, :])

            # ---- new_ws = clip(min(argmax - WL//2 + ws, ntok - WL), 0) ----
            mxs = spool.tile([B, 1], F32)
            nc.vector.tensor_reduce(mxs[:, :], score_sb[:, :],
                                    axis=mybir.AxisListType.X, op=OP.max)
            eq = wpool.tile([B, WL], F32)
            nc.vector.tensor_scalar(eq[:, :], score_sb[:, :], mxs[:, 0:1],
                                    None, OP.is_equal)
            sel = wpool.tile([B, WL], F32)
            nc.vector.tensor_tensor(sel[:, :], eq[:, :], revf[:, :],
                                    op=OP.mult)
            mrev = spool.tile([B, 1], F32)
            nc.vector.tensor_reduce(mrev[:, :], sel[:, :],
                                    axis=mybir.AxisListType.X, op=OP.max)
            # idx = T - mrev ; new = idx + ws - WL//2 = (T - WL//2 - mrev) + ws
            t1 = spool.tile([B, 1], F32)
            nc.vector.tensor_scalar(t1[:, :], mrev[:, :], -1.0,
                                    float(T - WL // 2), OP.mult, OP.add)
            wsf = spool.tile([B, 1], F32)
            nc.vector.tensor_copy(wsf[:, :], wscol[:, :])
            ntf = spool.tile([B, 1], F32)
            nc.vector.tensor_copy(ntf[:, :], ntcol[:, :])
            nwf = spool.tile([B, 1], F32)
            nc.vector.tensor_tensor(nwf[:, :], t1[:, :], wsf[:, :], op=OP.add)
            ntm = spool.tile([B, 1], F32)
            nc.vector.tensor_scalar(ntm[:, :], ntf[:, :], float(WL), None,
                                    OP.subtract)
            nw2 = spool.tile([B, 1], F32)
            nc.vector.tensor_tensor(nw2[:, :], nwf[:, :], ntm[:, :], op=OP.min)
            nw3 = spool.tile([B, 1], F32)
            nc.vector.tensor_scalar(nw3[:, :], nw2[:, :], 0.0, None, OP.max)
            nwi = spool.tile([B, 1], I32)
            nc.vector.tensor_copy(nwi[:, :], nw3[:, :])
            nc.sync.dma_start(new_ws.ap(), nwi[:, :])

    nc.compile()
    return nc


_NC = None


def _get_program():
    global _NC
    if _NC is None:
        _NC = build_program()
    return _NC


def build_in_maps(tokens, tokens_mask, num_tokens, query, cumulative_alignment,
                  initial_cumulative_alignment, window_start, conv_w, conv_b,
                  wq, bq, ws):
    tokens = np.asarray(tokens, dtype=np.float32)
    mask = np.asarray(tokens_mask)
    ca = np.where(mask, np.asarray(cumulative_alignment, dtype=np.float32),
                  np.float32(0.0)).astype(np.float32)
    num_tokens = np.asarray(num_tokens).astype(np.int32)
    window_start = np.asarray(window_start).astype(np.int32)
    query0 = np.asarray(query, dtype=np.float32)[0]          # [64, QD]
    init = np.asarray(initial_cumulative_alignment, dtype=np.float32)
    conv_wT = np.ascontiguousarray(np.asarray(conv_w, dtype=np.float32)[:, 0, :].T)
    qb = (np.asarray(conv_b, dtype=np.float32)
          + np.asarray(bq, dtype=np.float32)).astype(np.float32)
    wqT_np = np.ascontiguousarray(np.asarray(wq, dtype=np.float32).T)
    ws_np = np.asarray(ws, dtype=np.float32)

    tokens_bt = np.ascontiguousarray(tokens.transpose(1, 0, 2))  # [64, T, H]
    queryT_np = np.ascontiguousarray(query0.T)                   # [QD, 64]

    in_maps = []
    for c in range(NCORES):
        s = slice(c * B, (c + 1) * B)
        in_maps.append({
            "tokens": np.ascontiguousarray(tokens_bt[s]),
            "cum_in": np.ascontiguousarray(ca[s]),
            "init_ca": np.ascontiguousarray(init[s]),
            "wstart": np.ascontiguousarray(window_start[s]),
            "ntok": np.ascontiguousarray(num_tokens[s]),
            "queryT": np.ascontiguousarray(queryT_np[:, s]),
            "wqT": wqT_np,
            "convwT": conv_wT,
            "qbias": qb,
            "wsvec": ws_np,
        })
    return in_maps


def _unshard(results):
    context = np.concatenate([r["context"] for r in results], axis=0)
    cum = np.concatenate([r["cum_out"] for r in results], axis=0)
    align_full = np.concatenate([r["align_full"] for r in results], axis=0)
    new_ws = np.concatenate([np.asarray(r["new_ws"]).reshape(B)
                             for r in results], axis=0).astype(np.int32)
    return context, cum, align_full, new_ws


def kernel(**inputs):
    from concourse.bass_utils import run_bass_kernel_spmd

    in_maps = build_in_maps(**inputs)
    nc = _get_program()
    results = run_bass_kernel_spmd(nc, in_maps,
                                   core_ids=list(range(NCORES))).results
    return _unshard(results)


def kernel_profiled(**inputs):
    from concourse.bass_utils import run_bass_kernel_spmd

    in_maps = build_in_maps(**inputs)
    nc = _get_program()
    return run_bass_kernel_spmd(nc, in_maps, core_ids=list(range(NCORES)),
                                trace=True)
